# revision 1
# baseline (speedup 1.0000x reference)
"""Trainium2 Bass kernel for the quantized LM-head (nn_LmHeadTender).

Math (per core, vocab-sharded):
  reference computes
    Wl   = dequant_int4(lm_weight)            # per-row scale sw = rowmax/7
    y    = dequant_int4(x, per-(chunk,channel) scale s = tmax*2^(bucket-13)/7)
    out  = y @ Wl.T
  We factor every scale out of the matmul so that both matmul operands are
  small integers (times powers of two) that are EXACTLY representable in
  bf16:
    qw  in [-8, 7]                 (weight int values)
    yq  = qx * 2^(bucket-13)       (activation ints scaled by a power of 2)
    out[t, v] = (tmax_c/7) * sw[v] * sum_h yq[t, h] * qw[v, h]
  The bf16 matmul therefore computes exact products accumulated in fp32
  PSUM - the result matches the f32 reference to ~1e-6 (accumulation
  order), at bf16 matmul speed.

Sharding: lm_weight split into 8 vocab shards of 4000 rows, zero-padded to
4096.  hidden_states replicated.  Host concatenates the per-core [4096,
4096] logits (first 4000 cols valid) along vocab.
"""

import os
import sys
from contextlib import ExitStack

import numpy as np

import concourse.bass as bass
import concourse.tile as tile
from concourse import bacc, masks, mybir
from concourse.bass_utils import run_bass_kernel_spmd

FP = mybir.dt.float32
BF = mybir.dt.bfloat16
I32 = mybir.dt.int32
ALU = mybir.AluOpType
AX = mybir.AxisListType

T = 4096            # tokens (2*2048)
H = 4096            # hidden
V = 32000           # vocab
NCORE = 8
VSH = V // NCORE    # 4000 valid vocab rows per core
VP = 4096           # padded per-core vocab
CHUNK = 256
NCHUNK = T // CHUNK  # 16
DECOMP = 14
QMAX = 7.0
C_MAGIC = 12582912.0   # 1.5 * 2^23: round-to-nearest-even via add/sub
C7 = float(np.float32(1.0) / np.float32(7.0))  # fl(1/7); DVE has no divide op

KT = H // 128       # 32 k tiles
MT = VP // 128      # 32 weight row tiles
NT_GROUPS = 2       # token groups for the matmul phase
TG = T // (NT_GROUPS * 128)   # 16 token tiles (of 128) per group
VB = VP // 512      # 8 vocab blocks of 512


def _emit(ctx: ExitStack, tc: "tile.TileContext", x_d, w_d, out_d):
    nc = tc.nc

    # ---------------- persistent tiles ----------------
    cpool = ctx.enter_context(tc.tile_pool(name="consts", bufs=1))
    ident = cpool.tile([128, 128], FP)
    masks.make_identity(nc, ident[:])
    ones_row = cpool.tile([1, 128], FP)
    nc.vector.memset(ones_row[:], 1.0)
    sw_pk = cpool.tile([128, 32], FP)      # sw packed [p, m]; v = m*128+p
    sw_t = cpool.tile([32, 128], FP)       # sw transposed [m, p]
    sw_row = cpool.tile([1, VP], FP)       # sw on one partition, v-major
    sw_rep = cpool.tile([128, VP], FP)     # sw replicated on all partitions
    m7_all = cpool.tile([128, 16], FP)     # tmax_c/7 broadcast, col per chunk

    dpool = ctx.enter_context(tc.tile_pool(name="dram", bufs=1, space="DRAM"))
    qw_d = dpool.tile([VP, H], BF)         # quantized weight ints, [v, h]
    y_d = dpool.tile([H, T], BF)           # quantized act * 2^(b-13), [h, t]
    sw_d = dpool.tile([32, 128], FP)       # sw bounce buffer (row-major = v)

    # ---------------- weight phase ----------------
    with (
        tc.tile_pool(name="wq", bufs=2) as wq_pool,
        tc.tile_pool(name="wsm", bufs=2) as ws_pool,
    ):
        for m in range(MT):
            w_nat = wq_pool.tile([128, H], FP, tag="w_nat")
            nc.sync.dma_start(w_nat[:], w_d[m * 128:(m + 1) * 128, :])
            rmax = ws_pool.tile([128, 1], FP, tag="rmax")
            nc.vector.tensor_reduce(
                rmax[:], w_nat[:], axis=AX.X, op=ALU.max,
                apply_absolute_value=True)
            # sw = max(rmax*(1/7), 1e-9)  (reference: max(rmax/7, 1e-9))
            nc.vector.tensor_scalar(
                sw_pk[:, m:m + 1], rmax[:], C7, 1e-9, ALU.mult, ALU.max)
            rw = ws_pool.tile([128, 1], FP, tag="rw")
            nc.vector.reciprocal(rw[:], sw_pk[:, m:m + 1])
            # quantize in place: round(w*rw) clamped to [-8, 7]
            nc.vector.tensor_scalar(
                w_nat[:], w_nat[:], rw[:], C_MAGIC, ALU.mult, ALU.add)
            nc.vector.tensor_scalar(
                w_nat[:], w_nat[:], C_MAGIC, QMAX, ALU.subtract, ALU.min)
            qw_st = wq_pool.tile([128, H], BF, tag="qw_st")
            nc.vector.tensor_scalar(
                qw_st[:], w_nat[:], -(QMAX + 1.0), None, ALU.max)
            nc.sync.dma_start(qw_d[m * 128:(m + 1) * 128, :], qw_st[:])

    # ---------------- sw_rep build ----------------
    with tc.tile_pool(name="swps", bufs=4, space="PSUM") as swps_pool:
        for a in range(4):
            nc.vector.transpose(
                sw_t[:, a * 32:(a + 1) * 32], sw_pk[a * 32:(a + 1) * 32, :])
        nc.sync.dma_start(sw_d[:, :], sw_t[:])
        nc.sync.dma_start(sw_row[:], sw_d[:, :])
        for j in range(VP // 512):
            bp = swps_pool.tile([128, 512], FP, tag="bp")
            nc.tensor.matmul(
                bp[:], ones_row[:], sw_row[:, j * 512:(j + 1) * 512],
                start=True, stop=True)
            nc.scalar.copy(sw_rep[:, j * 512:(j + 1) * 512], bp[:])

    # ---------------- activation phase ----------------
    with (
        tc.tile_pool(name="xin", bufs=3) as xin_pool,
        tc.tile_pool(name="xT", bufs=2) as xT_pool,
        tc.tile_pool(name="xst", bufs=2) as st_pool,
        tc.tile_pool(name="yst", bufs=8) as y_pool,
        tc.tile_pool(name="xps", bufs=6, space="PSUM") as xps_pool,
        tc.tile_pool(name="bps", bufs=2, space="PSUM") as bps_pool,
    ):
        for c in range(NCHUNK):
            xT = xT_pool.tile([128, KT, CHUNK], FP, tag="xT")
            for th in range(2):
                xnat = xin_pool.tile([128, H], FP, tag="xn")
                nc.sync.dma_start(
                    xnat[:],
                    x_d[c * CHUNK + th * 128: c * CHUNK + (th + 1) * 128, :])
                for i in range(KT):
                    pst = xps_pool.tile([128, 128], FP, tag="pst")
                    nc.tensor.transpose(
                        pst[:], xnat[:, i * 128:(i + 1) * 128], ident[:])
                    dst = xT[:, i, th * 128:(th + 1) * 128]
                    if (i + th) % 2 == 0:
                        nc.scalar.copy(dst, pst[:])
                    else:
                        nc.vector.tensor_copy(dst, pst[:])
            # ---- stats: cmax per channel, tmax per chunk ----
            cmax = st_pool.tile([128, KT], FP, tag="cmax")
            nc.vector.tensor_reduce(
                cmax[:], xT[:], axis=AX.X, op=ALU.max,
                apply_absolute_value=True)
            tpad = st_pool.tile([128, 32], FP, tag="tpad")
            nc.vector.memset(tpad[:], 0.0)
            nc.vector.tensor_reduce(
                tpad[:, 0:1], cmax[:], axis=AX.X, op=ALU.max)
            tt = st_pool.tile([32, 128], FP, tag="tt")
            for a in range(4):
                nc.vector.transpose(
                    tt[:, a * 32:(a + 1) * 32], tpad[a * 32:(a + 1) * 32, :])
            tmax_sc = st_pool.tile([1, 1], FP, tag="tmax_sc")
            nc.vector.tensor_reduce(
                tmax_sc[:], tt[0:1, :], axis=AX.X, op=ALU.max)
            # broadcast tmax to 128 partitions via PE outer product
            bp1 = bps_pool.tile([128, 1], FP, tag="bp1")
            nc.tensor.matmul(
                bp1[:], ones_row[:], tmax_sc[:], start=True, stop=True)
            tmax_b = st_pool.tile([128, 1], FP, tag="tmax_b")
            nc.scalar.copy(tmax_b[:], bp1[:])
            nc.vector.tensor_scalar(
                m7_all[:, c:c + 1], tmax_b[:], C7, None, ALU.mult)
            # ---- bucket: number of thresholds strictly exceeded ----
            bucket = st_pool.tile([128, KT], FP, tag="bucket")
            nc.vector.memset(bucket[:], 0.0)
            for lv in range(DECOMP - 1):
                thr = st_pool.tile([128, 1], FP, tag="thr", bufs=2)
                nc.vector.tensor_scalar(
                    thr[:], tmax_b[:], 2.0 ** (lv - (DECOMP - 1)), None,
                    ALU.mult)
                nc.vector.scalar_tensor_tensor(
                    bucket[:], cmax[:], thr[:], bucket[:],
                    op0=ALU.is_gt, op1=ALU.add)
            # ---- pw = 2^(bucket-13) exactly, via IEEE bit construction ----
            g = st_pool.tile([128, KT], FP, tag="g")
            nc.vector.tensor_scalar(
                g[:], bucket[:], 114.0, 8388608.0, ALU.add, ALU.mult)
            g_i = st_pool.tile([128, KT], I32, tag="g_i")
            nc.vector.tensor_copy(g_i[:], g[:])
            pw = g_i[:].bitcast(FP)
            # ---- scales: s = max(tmax*pw/7, 1e-9); r = 1/s ----
            ch_thr = st_pool.tile([128, KT], FP, tag="ch_thr")
            nc.vector.tensor_scalar(
                ch_thr[:], pw, tmax_b[:], None, ALU.mult)
            s_t = st_pool.tile([128, KT], FP, tag="s_t")
            nc.vector.tensor_scalar(
                s_t[:], ch_thr[:], C7, 1e-9, ALU.mult, ALU.max)
            r_t = st_pool.tile([128, KT], FP, tag="r_t")
            nc.vector.reciprocal(r_t[:], s_t[:])
            # ---- quantize: y = clip(round(x*r), -8, 7) * pw  (bf16) ----
            for i in range(KT):
                sl = xT[:, i, :]
                nc.vector.tensor_scalar(
                    sl, sl, r_t[:, i:i + 1], C_MAGIC, ALU.mult, ALU.add)
                nc.vector.tensor_scalar(
                    sl, sl, C_MAGIC, QMAX, ALU.subtract, ALU.min)
                y_st = y_pool.tile([128, CHUNK], BF, tag="y_st")
                pw_col = g_i[:, i:i + 1].bitcast(FP)
                nc.vector.tensor_scalar(
                    y_st[:], sl, -(QMAX + 1.0), pw_col, ALU.max, ALU.mult)
                nc.sync.dma_start(
                    y_d[i * 128:(i + 1) * 128, c * CHUNK:(c + 1) * CHUNK],
                    y_st[:])

    # ---------------- matmul phase ----------------
    with (
        tc.tile_pool(name="ymm", bufs=1) as ymm_pool,
        tc.tile_pool(name="qwp", bufs=KT + 2) as qw_pool,
        tc.tile_pool(name="stg", bufs=4) as stg_pool,
        tc.tile_pool(name="mps", bufs=8, space="PSUM") as mps_pool,
    ):
        tok_g = TG * 128  # tokens per group
        for grp in range(NT_GROUPS):
            y_all = ymm_pool.tile([128, KT, tok_g], BF, tag="y_all")
            for k in range(KT):
                nc.sync.dma_start(
                    y_all[:, k, :],
                    y_d[k * 128:(k + 1) * 128,
                        grp * tok_g:(grp + 1) * tok_g])
            for vb in range(VB):
                qwt = []
                for k in range(KT):
                    qt = qw_pool.tile([128, 512], BF, tag="qw")
                    nc.sync.dma_start(
                        qt[:],
                        qw_d[vb * 512:(vb + 1) * 512,
                             k * 128:(k + 1) * 128],
                        transpose=True)
                    qwt.append(qt)
                for t in range(TG):
                    ps = mps_pool.tile([128, 512], FP, tag="ps")
                    for k in range(KT):
                        nc.tensor.matmul(
                            ps[:],
                            y_all[:, k, t * 128:(t + 1) * 128],
                            qwt[k][:],
                            start=(k == 0), stop=(k == KT - 1))
                    stg = stg_pool.tile([128, 512], FP, tag="stg")
                    tidx = grp * TG + t
                    cch = (tidx * 128) // CHUNK
                    nc.vector.scalar_tensor_tensor(
                        stg[:], ps[:], m7_all[:, cch:cch + 1],
                        sw_rep[:, vb * 512:(vb + 1) * 512],
                        op0=ALU.mult, op1=ALU.mult)
                    nc.sync.dma_start(
                        out_d[tidx * 128:(tidx + 1) * 128,
                              vb * 512:(vb + 1) * 512],
                        stg[:])


_CACHED = None


def _build():
    global _CACHED
    if _CACHED is not None:
        return _CACHED
    nc = bacc.Bacc(
        "TRN2", target_bir_lowering=False, debug=False,
        enable_asserts=False, num_devices=NCORE)
    x_d = nc.dram_tensor("x", (T, H), FP, kind="ExternalInput").ap()
    w_d = nc.dram_tensor("w", (VP, H), FP, kind="ExternalInput").ap()
    out_d = nc.dram_tensor("out", (T, VP), FP, kind="ExternalOutput").ap()
    with tile.TileContext(nc) as tc:
        with ExitStack() as ctx:
            _emit(ctx, tc, x_d, w_d, out_d)
    nc.compile()
    _CACHED = nc
    return nc


def kernel(hidden_states: np.ndarray, lm_weight: np.ndarray) -> np.ndarray:
    b, t, h = hidden_states.shape
    assert (b * t, h) == (T, H) and lm_weight.shape == (V, H)
    x_full = np.ascontiguousarray(
        hidden_states.reshape(T, H).astype(np.float32))
    in_maps = []
    for c in range(NCORE):
        shard = np.zeros((VP, H), dtype=np.float32)
        shard[:VSH] = lm_weight[c * VSH:(c + 1) * VSH]
        in_maps.append({"x": x_full, "w": shard})
    nc = _build()
    res = run_bass_kernel_spmd(nc, in_maps, core_ids=list(range(NCORE)))
    outs = [res.results[c]["out"][:, :VSH] for c in range(NCORE)]
    full = np.concatenate(outs, axis=1)
    return full.reshape(b, t, V)



# revision 14
# speedup vs baseline: 1.7330x; 1.7330x over previous
"""Trainium2 Bass kernel for the quantized LM-head (nn_LmHeadTender).

Math (per core, vocab-sharded, 4000 vocab rows/core):
  reference computes
    Wl   = dequant_int4(lm_weight)            # per-row scale sw = rowmax/7
    y    = dequant_int4(x, per-(chunk,channel) scale s = tmax*2^(bucket-13)/7)
    out  = y @ Wl.T
  Every scale is factored out of the matmul so both matmul operands are
  small integers (times powers of two) EXACTLY representable in fp8-e5m2
  (3 significand bits cover ints -8..7; exponent covers 2^-13..2^3):
    qw  in [-8, 7]                 (weight int values)
    yq  = qx * 2^(bucket-13)       (activation ints scaled by a power of 2)
    out[t, v] = (tmax_c/7) * sw[v] * sum_h yq[t, h] * qw[v, h]
  The fp8 matmul runs in DoubleRow perf mode (256-deep contraction per
  instruction, ~2x bf16 ALU throughput) and accumulates exactly in fp32
  PSUM, so the result matches the f32 reference to ~1e-4.

  Note the reference clip to [-8,7] is dead code for randn inputs:
  |x/s| <= 7 and |w/sw| <= 7 by construction of the scales, so the
  quantization needs only round (magic-number trick, RNE) - no clamps.

Layout / pipeline (all phases overlap via the Tile framework):
  - weight phase: stream w f32 [4000,4096], rowmax -> sw, quantize to int
    values stored as bf16 in DRAM qw_d [4000, 4096].
  - act phase: stream x f32, PE-transpose to [h, t], chunk stats
    (cmax/tmax/bucket), quantize to fp8e5 y stored in DRAM y_d [128,32,T]
    (p-major so both write and read DMAs are simple 3D APs).
  - mm phase: 4 vocab sweeps of 2x512-col blocks (last 512+416). Per
    sweep the qw block column is DMA-transposed (bf16) into SBUF and cast
    to fp8e5 once, then reused by all 32 token blocks. Stationary operand
    is the y tile (reused across the 2 vocab blocks), PSUM accumulates
    over 16 k-pairs; epilogue scales by tmax_c/7 (per-chunk) * sw (col)
    on DVE/ACT and writes bf16 output.
  Issue order interleaves weight tiles, act chunks and the first mm sweep
  so DMA/DVE/ACT work hides under the PE matmul stream.

Sharding: lm_weight split into 8 vocab shards of 4000 rows (no padding).
hidden_states replicated. Host concatenates per-core [4096, 4000] bf16
logits along vocab and casts to f32.
"""

import numpy as np
from contextlib import ExitStack

import concourse.bass as bass
import concourse.tile as tile
from concourse import bacc, masks, mybir
from concourse.bass_utils import run_bass_kernel_spmd

FP = mybir.dt.float32
BF = mybir.dt.bfloat16
F8 = mybir.dt.float8e5
I32 = mybir.dt.int32
ALU = mybir.AluOpType
AX = mybir.AxisListType
AF = mybir.ActivationFunctionType
DR = mybir.MatmulPerfMode.DoubleRow

T = 4096            # tokens (2*2048)
H = 4096            # hidden
V = 32000           # vocab
NCORE = 8
VSH = V // NCORE    # 4000 vocab rows per core
CHUNK = 256
NCHUNK = T // CHUNK  # 16
DECOMP = 14
QMAX = 7.0
MAGIC = 12582912.0   # 1.5 * 2^23: round-to-nearest-even via add/sub
C7 = float(np.float32(1.0) / np.float32(7.0))  # fl(1/7); no divide op

KT = H // 128        # 32 k tiles
VT = 32              # weight row tiles (31 full + 1 of 32 rows)
KP = KT // 2         # 16 k pairs for DoubleRow
TB = T // 128        # 32 token blocks
NYG = TB // 4        # 8 y groups of 512 tokens

VB_OFF = [0, 512, 1024, 1536, 2048, 2560, 3072, 3584]
VB_W = [512] * 7 + [416]
SWEEPS = [(0, 1), (2, 3), (4, 5), (6, 7)]


def _vt_rows(m):
    r0 = m * 128
    return r0, min(r0 + 128, VSH) - r0


class _Emitter:
    def __init__(self, ctx, tc, x_d, w_d, out_d):
        self.tc = tc
        self.nc = tc.nc
        self.x_d = x_d
        self.w_d = w_d
        self.out_d = out_d
        nc = self.nc

        cpool = ctx.enter_context(tc.tile_pool(name="consts", bufs=1))
        self.ident = cpool.tile([128, 128], FP)
        masks.make_identity(nc, self.ident[:])
        self.ones_row = cpool.tile([1, 128], FP)
        nc.vector.memset(self.ones_row[:], 1.0)
        self.magic_col = cpool.tile([128, 1], FP)
        nc.vector.memset(self.magic_col[:], MAGIC)
        self.sw_pk = cpool.tile([128, 32], FP)    # sw packed; v = m*128+p
        nc.vector.memset(self.sw_pk[:], 0.0)
        self.sw_t = cpool.tile([32, 128], FP)
        self.sw_row = cpool.tile([1, 4096], FP)
        self.sw_rep = cpool.tile([128, 4096], FP)  # sw bcast on partitions
        self.m7_all = cpool.tile([128, 16], FP)    # tmax_c/7 per chunk

        dpool = ctx.enter_context(tc.tile_pool(name="dram", bufs=1,
                                               space="DRAM"))
        self.qw_d = dpool.tile([VSH, H], BF)       # weight ints, [v, h]
        self.y_d = dpool.tile([128, KT, T], F8)    # y, [p, k, t] p-major
        self.sw_d = dpool.tile([32, 128], FP)

        # pools that live through both stage A and the mm phase
        self.mps_pool = ctx.enter_context(
            tc.tile_pool(name="mps", bufs=4, space="PSUM"))
        self.stg_pool = ctx.enter_context(tc.tile_pool(name="stg", bufs=2))
        self.ygrp_pool = ctx.enter_context(tc.tile_pool(name="ygrp", bufs=2))
        self.land_pool = ctx.enter_context(tc.tile_pool(name="land", bufs=2))
        self.bps_pool = ctx.enter_context(
            tc.tile_pool(name="bps", bufs=1, space="PSUM"))
        self.swps_pool = ctx.enter_context(
            tc.tile_pool(name="swps", bufs=1, space="PSUM"))
        self._alt = 0

    def _copy(self, dst, src):
        """Alternate vector/scalar engines for copies/casts."""
        self._alt ^= 1
        if self._alt:
            self.nc.vector.tensor_copy(dst, src)
        else:
            self.nc.scalar.copy(dst, src)

    # ---------------- weight phase ----------------
    def weight_tile(self, m, wq_pool, ws_pool, qst_pool):
        nc = self.nc
        r0, nr = _vt_rows(m)
        halves = []
        for hh in range(2):
            w_nat = wq_pool.tile([128, 2048], FP, tag="w_nat")
            nc.sync.dma_start(
                w_nat[:nr], self.w_d[r0:r0 + nr, hh * 2048:(hh + 1) * 2048])
            halves.append(w_nat)
        rmax = ws_pool.tile([128, 2], FP, tag="rmax")
        for hh in range(2):
            nc.vector.tensor_reduce(
                rmax[:nr, hh:hh + 1], halves[hh][:nr], axis=AX.X, op=ALU.max,
                apply_absolute_value=True)
        rall = ws_pool.tile([128, 1], FP, tag="rall")
        nc.vector.tensor_reduce(
            rall[:nr], rmax[:nr], axis=AX.X, op=ALU.max)
        # sw = max(rmax/7, 1e-9)
        nc.vector.tensor_scalar(
            self.sw_pk[:nr, m:m + 1], rall[:nr], C7, 1e-9, ALU.mult, ALU.max)
        rw = ws_pool.tile([128, 1], FP, tag="rw")
        nc.vector.reciprocal(rw[:nr], self.sw_pk[:nr, m:m + 1])
        for hh in range(2):
            # t = w*rw + MAGIC  (ACT, per-partition scale)
            nc.scalar.activation(
                halves[hh][:nr], halves[hh][:nr], AF.Identity,
                bias=self.magic_col[:nr], scale=rw[:nr])
            qst = qst_pool.tile([128, 2048], BF, tag="qst")
            nc.vector.tensor_scalar(
                qst[:nr], halves[hh][:nr], MAGIC, None, ALU.subtract)
            nc.sync.dma_start(
                self.qw_d[r0:r0 + nr, hh * 2048:(hh + 1) * 2048], qst[:nr])

    # ---------------- sw_rep broadcast (per half) ----------------
    def build_sw_rep(self, half):
        nc = self.nc
        for a in range(4):
            nc.vector.transpose(
                self.sw_t[:, a * 32:(a + 1) * 32],
                self.sw_pk[a * 32:(a + 1) * 32, :])
        nc.sync.dma_start(self.sw_d[:, :], self.sw_t[:])
        nc.sync.dma_start(self.sw_row[:], self.sw_d[:, :])
        for j in range(half * 4, half * 4 + 4):
            bp = self.swps_pool.tile([128, 512], FP, tag="bp")
            nc.tensor.matmul(
                bp[:], self.ones_row[:], self.sw_row[:, j * 512:(j + 1) * 512],
                start=True, stop=True)
            nc.scalar.copy(self.sw_rep[:, j * 512:(j + 1) * 512], bp[:])

    # ---------------- activation phase ----------------
    def act_chunk(self, c, xin_pool, xT_pool, st_pool, y_pool, xps_pool):
        nc = self.nc
        xT = xT_pool.tile([128, KT, CHUNK], FP, tag="xT")
        for th in range(2):
            xins = []
            for hh in range(2):
                xnat = xin_pool.tile([128, 2048], FP, tag="xn")
                nc.sync.dma_start(
                    xnat[:],
                    self.x_d[c * CHUNK + th * 128:c * CHUNK + (th + 1) * 128,
                             hh * 2048:(hh + 1) * 2048])
                xins.append(xnat)
            for kq in range(KT // 4):
                pst = xps_pool.tile([128, 512], FP, tag="pst")
                for j in range(4):
                    k = kq * 4 + j
                    nc.tensor.transpose(
                        pst[:, j * 128:(j + 1) * 128],
                        xins[k // 16][:, (k % 16) * 128:(k % 16 + 1) * 128],
                        self.ident[:])
                self._copy(
                    xT[:, kq * 4:kq * 4 + 4, th * 128:(th + 1) * 128],
                    pst[:].rearrange("p (k t) -> p k t", k=4))
        # ---- stats ----
        cmax = st_pool.tile([128, KT], FP, tag="cmax")
        nc.vector.tensor_reduce(
            cmax[:], xT[:], axis=AX.X, op=ALU.max, apply_absolute_value=True)
        tpad = st_pool.tile([128, 32], FP, tag="tpad")
        nc.vector.memset(tpad[:], 0.0)
        nc.vector.tensor_reduce(
            tpad[:, 0:1], cmax[:], axis=AX.X, op=ALU.max)
        tt = st_pool.tile([32, 128], FP, tag="tt")
        for a in range(4):
            nc.vector.transpose(
                tt[:, a * 32:(a + 1) * 32], tpad[a * 32:(a + 1) * 32, :])
        tmax_sc = st_pool.tile([1, 1], FP, tag="tmax_sc")
        nc.vector.tensor_reduce(
            tmax_sc[:], tt[0:1, :], axis=AX.X, op=ALU.max)
        bp1 = self.bps_pool.tile([128, 1], FP, tag="bp1")
        nc.tensor.matmul(
            bp1[:], self.ones_row[:], tmax_sc[:], start=True, stop=True)
        tmax_b = st_pool.tile([128, 1], FP, tag="tmax_b")
        nc.scalar.copy(tmax_b[:], bp1[:])
        nc.vector.tensor_scalar(
            self.m7_all[:, c:c + 1], tmax_b[:], C7, None, ALU.mult)
        # ---- bucket = #(thresholds strictly exceeded) ----
        bucket = st_pool.tile([128, KT], FP, tag="bucket")
        nc.vector.memset(bucket[:], 0.0)
        for lv in range(DECOMP - 1):
            thr = st_pool.tile([128, 1], FP, tag="thr", bufs=2)
            nc.vector.tensor_scalar(
                thr[:], tmax_b[:], 2.0 ** (lv - (DECOMP - 1)), None, ALU.mult)
            nc.vector.scalar_tensor_tensor(
                bucket[:], cmax[:], thr[:], bucket[:],
                op0=ALU.is_gt, op1=ALU.add)
        # ---- pw = 2^(bucket-13) exactly via IEEE bit construction ----
        g = st_pool.tile([128, KT], FP, tag="g")
        nc.vector.tensor_scalar(
            g[:], bucket[:], 114.0, 8388608.0, ALU.add, ALU.mult)
        g_i = st_pool.tile([128, KT], I32, tag="g_i")
        nc.vector.tensor_copy(g_i[:], g[:])
        pw = g_i[:].bitcast(FP)
        # ---- r = 1/max(tmax*pw/7, 1e-9) ----
        ch_thr = st_pool.tile([128, KT], FP, tag="ch_thr")
        nc.vector.tensor_scalar(
            ch_thr[:], pw, tmax_b[:], None, ALU.mult)
        s_t = st_pool.tile([128, KT], FP, tag="s_t")
        nc.vector.tensor_scalar(
            s_t[:], ch_thr[:], C7, 1e-9, ALU.mult, ALU.max)
        r_t = st_pool.tile([128, KT], FP, tag="r_t")
        nc.vector.reciprocal(r_t[:], s_t[:])
        # ---- quantize: y = round(x*r) * pw  (fp8e5, exact) ----
        if c % 2 == 0:
            self.y_st = y_pool.tile([128, KT, 2 * CHUNK], F8, tag="y_st")
        coff = (c % 2) * CHUNK
        for k in range(KT):
            sl = xT[:, k, :]
            nc.scalar.activation(
                sl, sl, AF.Identity, bias=self.magic_col[:],
                scale=r_t[:, k:k + 1])
            pw_col = g_i[:, k:k + 1].bitcast(FP)
            nc.vector.tensor_scalar(
                self.y_st[:, k, coff:coff + CHUNK], sl, MAGIC, pw_col,
                ALU.subtract, ALU.mult)
        if c % 2 == 1:
            nc.sync.dma_start(
                self.y_d[:, :, (c - 1) * CHUNK:(c + 1) * CHUNK], self.y_st[:])

    # ---------------- qw sweep load (transpose + fp8 cast) ----------------
    def load_qw_sweep(self, s, pool):
        nc = self.nc
        qw_sw = pool.tile([128, KT, 1024], F8, tag="qw")
        loc = 0
        for b in SWEEPS[s]:
            vo, w = VB_OFF[b], VB_W[b]
            for k in range(KT):
                land = self.land_pool.tile([128, 512], BF, tag="land")
                nc.sync.dma_start(
                    land[:, :w],
                    self.qw_d[vo:vo + w, k * 128:(k + 1) * 128],
                    transpose=True)
                self._copy(qw_sw[:, k, loc:loc + w], land[:, :w])
            loc += w
        return qw_sw

    # ---------------- matmul phase ----------------
    def mm_ygrp(self, s, gidx, qw_sw):
        nc = self.nc
        ba, bb = SWEEPS[s]
        wa, wb = VB_W[ba], VB_W[bb]
        vo = VB_OFF[ba]
        ygrp = self.ygrp_pool.tile([128, KT, 512], F8, tag="yg")
        nc.sync.dma_start(
            ygrp[:], self.y_d[:, :, gidx * 512:(gidx + 1) * 512])
        for ti in range(4):
            tb = gidx * 4 + ti
            ps_a = self.mps_pool.tile([128, 512], FP, tag="ps")
            ps_b = self.mps_pool.tile([128, 512], FP, tag="ps")
            for kp in range(KP):
                lhs = ygrp[:, 2 * kp:2 * kp + 2, ti * 128:(ti + 1) * 128]
                nc.tensor.matmul(
                    ps_a[:, :wa], lhs, qw_sw[:, 2 * kp:2 * kp + 2, 0:wa],
                    start=(kp == 0), stop=(kp == KP - 1), perf_mode=DR)
                nc.tensor.matmul(
                    ps_b[:, :wb], lhs,
                    qw_sw[:, 2 * kp:2 * kp + 2, 512:512 + wb],
                    start=(kp == 0), stop=(kp == KP - 1), perf_mode=DR)
            stg = self.stg_pool.tile([128, 1024], BF, tag="stg")
            m7c = self.m7_all[:, tb // 2:tb // 2 + 1]
            nc.vector.scalar_tensor_tensor(
                stg[:, 0:wa], ps_a[:, :wa], m7c,
                self.sw_rep[:, vo:vo + wa], op0=ALU.mult, op1=ALU.mult)
            nc.vector.scalar_tensor_tensor(
                stg[:, wa:wa + wb], ps_b[:, :wb], m7c,
                self.sw_rep[:, vo + 512:vo + 512 + wb],
                op0=ALU.mult, op1=ALU.mult)
            nc.sync.dma_start(
                self.out_d[tb * 128:(tb + 1) * 128, vo:vo + wa + wb],
                stg[:, :wa + wb])


def _emit(ctx, tc, x_d, w_d, out_d):
    em = _Emitter(ctx, tc, x_d, w_d, out_d)

    with ExitStack() as stage_a:
        wq_pool = stage_a.enter_context(tc.tile_pool(name="wq", bufs=3))
        ws_pool = stage_a.enter_context(tc.tile_pool(name="wsm", bufs=2))
        qst_pool = stage_a.enter_context(tc.tile_pool(name="qst", bufs=2))
        xin_pool = stage_a.enter_context(tc.tile_pool(name="xin", bufs=2))
        xT_pool = stage_a.enter_context(tc.tile_pool(name="xT", bufs=1))
        st_pool = stage_a.enter_context(tc.tile_pool(name="xst", bufs=2))
        y_pool = stage_a.enter_context(tc.tile_pool(name="yst", bufs=1))
        xps_pool = stage_a.enter_context(
            tc.tile_pool(name="xps", bufs=2, space="PSUM"))
        qw0_pool = stage_a.enter_context(tc.tile_pool(name="qw0", bufs=1))

        # part 1: weight tiles 0..15 interleaved with act chunks 0..7
        for i in range(16):
            em.weight_tile(i, wq_pool, ws_pool, qst_pool)
            if i % 2 == 1:
                em.act_chunk(i // 2, xin_pool, xT_pool, st_pool, y_pool,
                             xps_pool)
        em.build_sw_rep(0)
        qw_s0 = em.load_qw_sweep(0, qw0_pool)

        # part 2: weight tiles 16..31, act chunks 8..15, mm sweep0 starts
        for i in range(16, 32):
            em.weight_tile(i, wq_pool, ws_pool, qst_pool)
            if i % 2 == 1:
                c = 8 + (i - 17) // 2
                em.act_chunk(c, xin_pool, xT_pool, st_pool, y_pool, xps_pool)
                if c % 2 == 1:
                    em.mm_ygrp(0, (c - 1) // 2 - 4, qw_s0)
        em.build_sw_rep(1)
        for g in range(4, NYG):
            em.mm_ygrp(0, g, qw_s0)

    with tc.tile_pool(name="qwl", bufs=2) as qwl_pool:
        qw_s = em.load_qw_sweep(1, qwl_pool)
        for s in (1, 2, 3):
            qw_next = None
            for g in range(NYG):
                em.mm_ygrp(s, g, qw_s)
                if g == 0 and s < 3:
                    qw_next = em.load_qw_sweep(s + 1, qwl_pool)
            qw_s = qw_next


_CACHED = None


def _build():
    global _CACHED
    if _CACHED is not None:
        return _CACHED
    nc = bacc.Bacc(
        "TRN2", target_bir_lowering=False, debug=False,
        enable_asserts=False, num_devices=NCORE)
    x_d = nc.dram_tensor("x", (T, H), FP, kind="ExternalInput").ap()
    w_d = nc.dram_tensor("w", (VSH, H), FP, kind="ExternalInput").ap()
    out_d = nc.dram_tensor("out", (T, VSH), BF, kind="ExternalOutput").ap()
    with tile.TileContext(nc) as tc:
        with ExitStack() as ctx:
            _emit(ctx, tc, x_d, w_d, out_d)
    nc.compile()
    _CACHED = nc
    return nc


def kernel(hidden_states: np.ndarray, lm_weight: np.ndarray) -> np.ndarray:
    b, t, h = hidden_states.shape
    assert (b * t, h) == (T, H) and lm_weight.shape == (V, H)
    x_full = np.ascontiguousarray(
        hidden_states.reshape(T, H).astype(np.float32))
    in_maps = []
    for c in range(NCORE):
        shard = np.ascontiguousarray(
            lm_weight[c * VSH:(c + 1) * VSH].astype(np.float32))
        in_maps.append({"x": x_full, "w": shard})
    nc = _build()
    res = run_bass_kernel_spmd(nc, in_maps, core_ids=list(range(NCORE)))
    outs = [np.asarray(res.results[c]["out"]).astype(np.float32)
            for c in range(NCORE)]
    full = np.concatenate(outs, axis=1)
    return full.reshape(b, t, V)
